# revision 8
# baseline (speedup 1.0000x reference)
"""Self-contained kernel for nn_HTA_5677946766007 (sparse_attention).

Data-parallel over batch B=8 across 8 NeuronCores: one GSPMD-sharded
jit executable (leaner dispatch than jax.pmap here), one image per
core, weights replicated. Optimized for the Neuron XLA backend:
 - both conv_general_dilated calls removed (depthwise 3x3 -> 9 shifted
   multiply-adds; 8x8/s8 reduction conv -> reshape + dense matmul)
 - window attention computed over 2x2-window groups with a
   block-diagonal mask (4x fewer / larger matmuls; measured 2.7x
   faster on device than per-window batching)
 - matmuls and transposes run in bf16 (fp32 accumulation)
 - softmax without max-subtraction (scores are tiny by construction)
 - inputs placed with their exact shardings (avoids per-call reshard)
All shapes hardcoded from the problem spec.

Measured (rel err 1.303e-4 vs fp32 reference; gate 2e-2):
 - blocking per-call wall: 85-88 ms, of which ~81 ms is the axon
   client-tunnel sync round-trip (a no-op dispatch costs the same)
 - true device compute: ~4.8 ms/call (pipelined-dispatch measurement)
   vs ~24 ms implied for the original pmap/conv baseline (105.8 ms).
"""

import functools

import jax
import jax.numpy as jnp
import numpy as np

B, H, W, C = 8, 112, 112, 128
NH, WS, SR, HIDDEN = 4, 7, 8, 512
hd = C // NH
N = H * W
EPS = 1e-5
N_CORES = 8
bf16, f32 = jnp.bfloat16, jnp.float32


def _mm(a, b):
    return jax.lax.dot_general(a.astype(bf16), b.astype(bf16),
                               (((a.ndim - 1,), (0,)), ((), ())),
                               preferred_element_type=f32)


def _ln(x, g, b):
    mu = jnp.mean(x, -1, keepdims=True)
    xc = x - mu
    var = jnp.mean(xc * xc, -1, keepdims=True)
    return xc * jax.lax.rsqrt(var + EPS) * g + b


def _sm(s):  # softmax without max-subtraction (|s| << 1 by construction)
    e = jnp.exp(s)
    return e / jnp.sum(e, -1, keepdims=True)


GS = 2 * WS   # attention computed over 2x2-window groups (14x14 tokens)
_NG = H // GS  # 8 groups per axis -> 64 groups
_idx = np.arange(GS * GS)
_wi, _wj = _idx // GS, _idx % GS
_MASK_NP = (((_wi[:, None] // WS) == (_wi[None, :] // WS)) &
            ((_wj[:, None] // WS) == (_wj[None, :] // WS))).astype(np.float32)


def _local_attn(x, qkv_w, qkv_b, pw, pb):
    # 7x7 window attention via 2x2-window groups + block-diagonal mask:
    # 4x fewer / 4x larger batched matmuls, coarser transposes.
    # Measured 2.7x faster on device than per-window batching.
    g2 = GS * GS
    mask = jnp.asarray(_MASK_NP)
    qkv = (_mm(x, qkv_w) + qkv_b).astype(bf16)          # (N, 384)
    qkv = qkv.reshape(_NG, GS, _NG, GS, 3, NH, hd)
    qkv = qkv.transpose(4, 0, 2, 5, 1, 3, 6).reshape(3, _NG * _NG, NH, g2, hd)
    q, k, v = qkv[0], qkv[1], qkv[2]
    s = jnp.einsum('wnid,wnjd->wnij', q, k, preferred_element_type=f32) * (hd ** -0.5)
    e = jnp.exp(s) * mask   # |s| << 1, so exp never overflows
    attn = (e / jnp.sum(e, -1, keepdims=True)).astype(bf16)
    o = jnp.einsum('wnij,wnjd->wnid', attn, v, preferred_element_type=f32).astype(bf16)
    o = o.reshape(_NG, _NG, NH, GS, GS, hd).transpose(0, 3, 1, 4, 2, 5).reshape(N, C)
    return _mm(o, pw) + pb


def _mlp(x, f1w, f1b, dww, dwb, f2w, f2b):
    h = (_mm(x, f1w) + f1b).astype(bf16).reshape(H, W, HIDDEN)
    wk = dww.reshape(3, 3, HIDDEN).astype(bf16)
    hp = jnp.pad(h, ((1, 1), (1, 1), (0, 0)))
    acc = None
    for di in range(3):
        for dj in range(3):
            t = hp[di:di + H, dj:dj + W, :] * wk[di, dj]
            acc = t if acc is None else acc + t
    hi = acc.astype(f32) + dwb
    h = jax.nn.gelu(hi.reshape(N, HIDDEN), approximate=True)
    return _mm(h, f2w) + f2b


def _global_attn(x, qw, qb, kvw, kvb, srw, srb, sng, snb, pw, pb):
    M = (H // SR) * (W // SR)  # 196
    q = (_mm(x, qw) + qb).astype(bf16).reshape(N, NH, hd)
    xi = x.astype(bf16).reshape(H // SR, SR, W // SR, SR, C)
    xi = xi.transpose(0, 2, 1, 3, 4).reshape(M, SR * SR * C)
    xs = jax.lax.dot_general(xi, srw.reshape(SR * SR * C, C).astype(bf16),
                             (((1,), (0,)), ((), ())),
                             preferred_element_type=f32) + srb
    xs = _ln(xs, sng, snb)
    kv = (_mm(xs, kvw) + kvb).astype(bf16).reshape(M, 2, NH, hd)
    k, v = kv[:, 0], kv[:, 1]
    s = jnp.einsum('nhd,mhd->hnm', q, k, preferred_element_type=f32) * (hd ** -0.5)
    attn = _sm(s).astype(bf16)
    o = jnp.einsum('hnm,mhd->nhd', attn, v, preferred_element_type=f32)
    return _mm(o.reshape(N, C).astype(bf16), pw) + pb


def _forward(x, w):
    x = x + _local_attn(_ln(x, w['l_n1_g'], w['l_n1_b']),
                        w['l_qkv_w'], w['l_qkv_b'], w['l_pw'], w['l_pb'])
    x = x + _mlp(_ln(x, w['l_n2_g'], w['l_n2_b']),
                 w['l_f1w'], w['l_f1b'], w['l_dww'], w['l_dwb'],
                 w['l_f2w'], w['l_f2b'])
    x = x + _global_attn(_ln(x, w['g_n1_g'], w['g_n1_b']),
                         w['g_qw'], w['g_qb'], w['g_kvw'], w['g_kvb'],
                         w['g_srw'], w['g_srb'], w['g_sng'], w['g_snb'],
                         w['g_pw'], w['g_pb'])
    x = x + _mlp(_ln(x, w['g_n2_g'], w['g_n2_b']),
                 w['g_f1w'], w['g_f1b'], w['g_dww'], w['g_dwb'],
                 w['g_f2w'], w['g_f2b'])
    return x


@functools.cache
def _jit_ctx():
    # GSPMD-sharded jit over the batch dim: one partitioned executable
    # across the 8 cores. Measurably leaner dispatch than jax.pmap here.
    devs = jax.devices()[:N_CORES]
    mesh = jax.sharding.Mesh(np.array(devs), ('x',))
    sh = jax.sharding.NamedSharding(mesh, jax.sharding.PartitionSpec('x'))
    rep = jax.sharding.NamedSharding(mesh, jax.sharding.PartitionSpec())
    fn = jax.jit(jax.vmap(_forward, in_axes=(0, None)),
                 in_shardings=(sh, rep), out_shardings=sh)
    return fn, sh, rep


def _pmapped():  # name kept for test.py compatibility
    return _jit_ctx()[0]


def kernel(**inputs) -> np.ndarray:
    fn, sh, rep = _jit_ctx()
    x = np.ascontiguousarray(inputs['x'], dtype=np.float32)
    names = [k for k in inputs if k != 'x']
    arrs = [np.asarray(inputs[k], dtype=np.float32) for k in names]
    # single batched transfer instead of 35 sequential device_puts
    placed = jax.device_put(arrs + [x.reshape(B, N, C)],
                            [rep] * len(arrs) + [sh])
    w = dict(zip(names, placed[:-1]))
    out = fn(placed[-1], w)
    return np.asarray(out).astype(np.float32)
